# revision 3
# baseline (speedup 1.0000x reference)
"""AttentionPool (segment softmax-weighted mean pool) Trainium2 kernel.

Math (reference, fp32):
    h = relu(x @ W1 + b1); l = h @ W2 + b2
    w = exp(l - max(l))                       # global max shift
    mean_w = segment_mean(w, batch, B)        # (B, 1)
    denom = mean_w[batch] * N
    out = segment_mean(w * x / (denom + 1e-8), batch)   # (B, D)

The kernel skips the global max shift: with these magnitudes (l in
[-3, 3]) exp() cannot overflow, and the result is identical up to the
1e-8 epsilon term (relative perturbation ~1e-12), so no cross-core
communication is needed at all once nodes are sharded on segment
boundaries.

Sharding: batch is sorted, so core c owns segments [128c, 128(c+1)) and
the contiguous node range covering them (~62.5k nodes).

V2: all-bf16 dataplane. x is loaded twice in bf16 (d-major for the MLP
matmul, node-major + ones column for the segment matmul) = 130 KB per
128-node tile vs 212 KB in V1 (f32 node-major + bf16 d-major + fp8
host mask). The one-hot segment matrix is built on-device from a tiny
[128, T] f32 local-segment-index tensor with one tensor_scalar
(is_equal, then mult by e) per tile, alternating DVE / GpSimd.

Inputs per core (host-prepared):
  xe   (128, T*258) bf16 : node-major x tiles [x | 1 | pad], swizzled
  xb   (128, T*256) bf16 : d-major (transposed) x for the MLP matmul
  bvec (128, T)     f32  : local segment idx per node (999 for padding)

Per 512-node macro-tile (4 x 128-node tiles, per core), software-
pipelined so every engine's inputs were produced >= 1 macro earlier:
  - 1-2 DMAs (two macros per dma_start): xe 2x(128x1032) bf16,
    xb 2x(128x1024) bf16
  - PE mm1: Ht(128f x 512n) = W1.T @ Xb   (two K=128 chunks, PSUM acc)
  - ACT: Hr = relu(Ht + b1) -> SBUF bf16  (one op)
  - PE mm2 x4: l(128n x 2) = Hr_sub.T @ [W2|0] -> one LP (128 x 8)
  - ACT: e(128n x 4) = exp(l + b2) -> SBUF (one strided op)
  - DVE/Pool x4: A(128n x 128s) = (iota == bvec) * e  (one tensor_scalar)
  - PE mm_seg x4: S(128s x 258) += A_sub.T @ xe_tile (PSUM acc;
    xe col 256 is the ones column -> per-segment sum of e)
Tail: S -> SBUF, per-segment normalize with host 1/cnt terms, DMA out
(128 x 256) rows = segments [128c, 128(c+1)).
"""
import numpy as np
from contextlib import ExitStack

import ml_dtypes

import concourse.bass as bass
import concourse.bacc as bacc
import concourse.mybir as mybir
import concourse.tile as tile
from concourse.bass_utils import run_bass_kernel_spmd

F32 = mybir.dt.float32
BF16 = mybir.dt.bfloat16

NCORES = 8
D = 256
B = 1024
SEG_PER_CORE = B // NCORES  # 128
MAC = 4  # tiles per macro
DMAC = 2  # macros per dma_start


def build_nc(T, num_devices=NCORES):
    """Build the SPMD program for T 128-node tiles per core."""
    nc = bacc.Bacc("TRN2", target_bir_lowering=False, debug=False,
                   enable_asserts=False, num_devices=num_devices)

    xe_ext = nc.dram_tensor("xe", [128, T * 258], BF16, kind="ExternalInput")
    xb_ext = nc.dram_tensor("xb", [128, T * 256], BF16, kind="ExternalInput")
    w1_ext = nc.dram_tensor("w1", [256, 128], BF16, kind="ExternalInput")
    w2p_ext = nc.dram_tensor("w2p", [128, 2], BF16, kind="ExternalInput")
    b1_ext = nc.dram_tensor("b1c", [128, 1], F32, kind="ExternalInput")
    b2_ext = nc.dram_tensor("b2c", [128, 1], F32, kind="ExternalInput")
    bvec_ext = nc.dram_tensor("bvec", [128, T], F32, kind="ExternalInput")
    iota_ext = nc.dram_tensor("iota", [128, 128], BF16, kind="ExternalInput")
    inv1_ext = nc.dram_tensor("inv1c", [128, 1], F32, kind="ExternalInput")
    inv2_ext = nc.dram_tensor("inv2c", [128, 1], F32, kind="ExternalInput")
    out_ext = nc.dram_tensor("out", [128, 256], F32, kind="ExternalOutput")

    with tile.TileContext(nc) as tc, ExitStack() as ctx:
        const = ctx.enter_context(tc.tile_pool(name="const", bufs=1))
        xpool = ctx.enter_context(tc.tile_pool(name="xpool", bufs=4))
        xbpool = ctx.enter_context(tc.tile_pool(name="xbpool", bufs=4))
        hrp = ctx.enter_context(tc.tile_pool(name="hrp", bufs=5))
        ep = ctx.enter_context(tc.tile_pool(name="ep", bufs=5))
        ap_pool = ctx.enter_context(tc.tile_pool(name="ap", bufs=5))
        tailp = ctx.enter_context(tc.tile_pool(name="tailp", bufs=1))
        htps = ctx.enter_context(tc.tile_pool(name="htps", bufs=3,
                                              space="PSUM"))
        lps = ctx.enter_context(tc.tile_pool(name="lps", bufs=2, space="PSUM"))
        sps = ctx.enter_context(tc.tile_pool(name="sps", bufs=1, space="PSUM"))

        # --- constants ---
        w1a = const.tile([128, 128], BF16)
        nc.sync.dma_start(w1a[:], w1_ext[0:128, :])
        w1b = const.tile([128, 128], BF16)
        nc.sync.dma_start(w1b[:], w1_ext[128:256, :])
        w2p = const.tile([128, 2], BF16)
        nc.sync.dma_start(w2p[:], w2p_ext[:, :])
        b1c = const.tile([128, 1], F32)
        nc.sync.dma_start(b1c[:], b1_ext[:, :])
        b2c = const.tile([128, 1], F32)
        nc.sync.dma_start(b2c[:], b2_ext[:, :])
        iota = const.tile([128, 128], BF16)
        nc.sync.dma_start(iota[:], iota_ext[:, :])
        bvec = const.tile([128, T], F32)
        nc.sync.dma_start(bvec[:], bvec_ext[:, :])
        inv1c = const.tile([128, 1], F32)
        nc.sync.dma_start(inv1c[:], inv1_ext[:, :])
        inv2c = const.tile([128, 1], F32)
        nc.sync.dma_start(inv2c[:], inv2_ext[:, :])

        S_ps = sps.tile([128, 258], F32)

        assert T % (MAC * DMAC) == 0
        nmac = T // MAC
        total = nmac
        # Stage lags (in macro ticks) for software pipelining.
        LAG_MM1, LAG_MM2, LAG_SEG = 1, 2, 3
        state = {}
        for vi in range(total + LAG_SEG + 1):
            i = vi if vi < total else -1
            if vi < total:
                st = state.setdefault(vi, {})
                if i % DMAC == 0:
                    XM = xpool.tile([128, 258 * MAC * DMAC], BF16, tag="x")
                    nc.sync.dma_start(
                        XM[:], xe_ext[:, 258 * MAC * i:258 * MAC * (i + DMAC)])
                    XB = xbpool.tile([128, 256 * MAC * DMAC], BF16, tag="xb")
                    nc.sync.dma_start(
                        XB[:], xb_ext[:, 256 * MAC * i:256 * MAC * (i + DMAC)])
                    state["XM"], state["XB"] = XM, XB
                else:
                    XM, XB = state["XM"], state["XB"]
                h = (i % DMAC) * MAC
                st["X"] = [XM[:, 258 * (h + j):258 * (h + j + 1)]
                           for j in range(MAC)]
                st["XBv"] = XB[:, 1024 * (i % DMAC):1024 * (i % DMAC + 1)]

            k = vi - LAG_MM1
            if 0 <= k < total:
                st = state[k]
                xbv = st["XBv"]
                HT = htps.tile([128, 512], F32)
                nc.tensor.matmul(HT[:], w1a[:], xbv[:, 0:512],
                                 start=True, stop=False)
                nc.tensor.matmul(HT[:], w1b[:], xbv[:, 512:1024],
                                 start=False, stop=True)
                HR = hrp.tile([128, 512], BF16)
                nc.scalar.activation(HR[:], HT[:],
                                     mybir.ActivationFunctionType.Relu,
                                     bias=b1c[:])
                st["HR"] = HR

            k = vi - LAG_MM2
            if 0 <= k < total:
                st = state[k]
                HR = st["HR"]
                LP = lps.tile([128, 2 * MAC], F32)
                for j in range(MAC):
                    nc.tensor.matmul(LP[:, 2 * j:2 * j + 2],
                                     HR[:, 128 * j:128 * (j + 1)], w2p[:],
                                     start=True, stop=True,
                                     skip_group_check=True)
                E4 = ep.tile([128, MAC], F32)
                nc.scalar.activation(E4[:], LP[:, 0:2 * MAC:2],
                                     mybir.ActivationFunctionType.Exp,
                                     bias=b2c[:])
                A2 = ap_pool.tile([128, 128 * MAC], BF16, tag="A")
                for j in range(MAC):
                    eng = nc.vector if j % 2 == 0 else nc.gpsimd
                    eng.tensor_scalar(
                        out=A2[:, 128 * j:128 * (j + 1)],
                        in0=iota[:],
                        scalar1=bvec[:, MAC * k + j:MAC * k + j + 1],
                        scalar2=E4[:, j:j + 1],
                        op0=mybir.AluOpType.is_equal,
                        op1=mybir.AluOpType.mult)
                st["A2"] = A2

            k = vi - LAG_SEG
            if 0 <= k < total:
                st = state.pop(k)
                A2 = st["A2"]
                for j in range(MAC):
                    nc.tensor.matmul(S_ps[:, :],
                                     A2[:, 128 * j:128 * (j + 1)],
                                     st["X"][j][:, :],
                                     start=(k == 0 and j == 0),
                                     stop=(k == total - 1 and j == MAC - 1),
                                     skip_group_check=True)

        # ---- tail: normalize ----
        S_sb = tailp.tile([128, 258], F32)
        nc.vector.tensor_copy(S_sb[:], S_ps[:])
        dvec = tailp.tile([128, 1], F32)
        nc.vector.tensor_scalar(out=dvec[:], in0=S_sb[:, 256:257],
                                scalar1=inv1c[:], scalar2=1e-8,
                                op0=mybir.AluOpType.mult,
                                op1=mybir.AluOpType.add)
        rvec = tailp.tile([128, 1], F32)
        nc.vector.reciprocal(rvec[:], dvec[:])
        scl = tailp.tile([128, 1], F32)
        nc.vector.tensor_mul(scl[:], rvec[:], inv2c[:])
        out_sb = tailp.tile([128, 256], F32)
        nc.vector.tensor_scalar(out=out_sb[:], in0=S_sb[:, 0:256],
                                scalar1=scl[:], scalar2=None,
                                op0=mybir.AluOpType.mult)
        nc.sync.dma_start(out_ext[:, :], out_sb[:])

    nc.compile()
    return nc


def host_prep(x, batch, W1, b1, W2, b2, ncores=NCORES):
    """Shard on segment boundaries and build per-core input maps."""
    x = np.ascontiguousarray(np.asarray(x, dtype=np.float32))
    batch = np.asarray(batch).astype(np.int64)
    W1 = np.asarray(W1, dtype=np.float32)
    b1 = np.asarray(b1, dtype=np.float32)
    W2 = np.asarray(W2, dtype=np.float32)
    b2 = np.asarray(b2, dtype=np.float32)
    N = x.shape[0]

    sizes = np.bincount(batch, minlength=B)
    starts = np.zeros(B + 1, np.int64)
    starts[1:] = np.cumsum(sizes)
    spc = B // ncores

    T = 0
    for c in range(ncores):
        n = int(starts[spc * (c + 1)] - starts[spc * c])
        T = max(T, (n + 127) // 128)
    T += (-T) % (MAC * DMAC)

    w1_np = W1.astype(ml_dtypes.bfloat16)
    w2p_np = np.zeros((128, 2), ml_dtypes.bfloat16)
    w2p_np[:, 0] = W2[:, 0].astype(ml_dtypes.bfloat16)
    b1_np = b1.reshape(128, 1).astype(np.float32)
    b2_np = np.full((128, 1), float(np.asarray(b2).reshape(-1)[0]), np.float32)
    iota_np = np.ascontiguousarray(
        np.broadcast_to(np.arange(128, dtype=np.float32), (128, 128))
    ).astype(ml_dtypes.bfloat16)

    in_maps = []
    for c in range(ncores):
        lo, hi = int(starts[spc * c]), int(starts[spc * (c + 1)])
        n = hi - lo
        xb16 = x[lo:hi].astype(ml_dtypes.bfloat16)

        xe = np.zeros((T * 128, 258), ml_dtypes.bfloat16)
        xe[:n, 0:256] = xb16
        xe[:, 256] = 1.0
        # swizzle: partition p holds its own rows contiguously
        xe = np.ascontiguousarray(
            xe.reshape(T, 128, 258).transpose(1, 0, 2).reshape(128, T * 258))

        nm = T // MAC
        xbm = np.zeros((256, T * 128), ml_dtypes.bfloat16)
        xbm[:, :n] = xb16.T
        # (c p) (i n) -> p (i c n)
        xbm = np.ascontiguousarray(
            xbm.reshape(2, 128, nm, 512).transpose(1, 2, 0, 3).reshape(
                128, T * 256))

        local = (batch[lo:hi] - spc * c).astype(np.int64)
        assert n == 0 or (local.min() >= 0 and local.max() < spc)

        bvec = np.full((128, T), 999.0, np.float32)
        pos = np.arange(n)
        bvec[pos % 128, pos // 128] = local.astype(np.float32)

        cnt = sizes[spc * c: spc * (c + 1)].astype(np.float32)
        cntc = np.maximum(cnt, 1.0)
        inv1 = (np.float32(N) / cntc).reshape(128, 1).astype(np.float32)
        inv2 = (1.0 / cntc).reshape(128, 1).astype(np.float32)

        im = {
            "xe": xe,
            "xb": xbm,
            "w1": w1_np,
            "w2p": w2p_np,
            "b1c": b1_np,
            "b2c": b2_np,
            "bvec": bvec,
            "iota": iota_np,
            "inv1c": inv1,
            "inv2c": inv2,
        }
        in_maps.append(im)
    return T, in_maps


_NC_CACHE = {}
_LAST_RESULTS = None
RUN_KWARGS = {}


def kernel(x, batch, W1, b1, W2, b2):
    global _LAST_RESULTS
    import os
    T, in_maps = host_prep(x, batch, W1, b1, W2, b2)
    key = T
    if key not in _NC_CACHE:
        _NC_CACHE[key] = build_nc(T)
    nc = _NC_CACHE[key]
    kw = dict(RUN_KWARGS)
    if os.environ.get("BASS_KERNEL_TRACE"):
        kw.setdefault("trace", True)
    res = run_bass_kernel_spmd(nc, in_maps, list(range(NCORES)), **kw)
    _LAST_RESULTS = res
    out = np.concatenate([res.results[c]["out"] for c in range(NCORES)], axis=0)
    return out.astype(np.float32)


# revision 12
# speedup vs baseline: 2.8571x; 2.8571x over previous
"""AttentionPool (segment softmax-weighted mean pool) Trainium2 kernel.

Math (reference, fp32):
    h = relu(x @ W1 + b1); l = h @ W2 + b2
    w = exp(l - max(l))                       # global max shift
    mean_w = segment_mean(w, batch, B)        # (B, 1)
    denom = mean_w[batch] * N
    out = segment_mean(w * x / (denom + 1e-8), batch)   # (B, D)

The kernel skips the global max shift: with these magnitudes (l in
[-3, 3]) exp() cannot overflow, and the result is identical up to the
1e-8 epsilon term (relative perturbation ~1e-12), so no cross-core
communication is needed at all once nodes are sharded on segment
boundaries.

Sharding: batch is sorted, so core c owns segments [128c, 128(c+1)) and
the contiguous node range covering them (~62.5k nodes).

V2: all-bf16 dataplane. x is loaded twice in bf16 (d-major for the MLP
matmul, node-major + ones column for the segment matmul) = 130 KB per
128-node tile vs 212 KB in V1 (f32 node-major + bf16 d-major + fp8
host mask). The one-hot segment matrix is built on-device from a tiny
[128, T] f32 local-segment-index tensor with one tensor_scalar
(is_equal, then mult by e) per tile, alternating DVE / GpSimd.

Inputs per core (host-prepared):
  xe   (128, T*258) bf16 : node-major x tiles [x | 1 | pad], swizzled
  xb   (128, T*256) bf16 : d-major (transposed) x for the MLP matmul
  bvec (128, T)     f32  : local segment idx per node (999 for padding)

Per 512-node macro-tile (4 x 128-node tiles, per core), software-
pipelined so every engine's inputs were produced >= 1 macro earlier:
  - 1-2 DMAs (two macros per dma_start): xe 2x(128x1032) bf16,
    xb 2x(128x1024) bf16
  - PE mm1: Ht(128f x 512n) = W1.T @ Xb   (two K=128 chunks, PSUM acc)
  - ACT: Hr = relu(Ht + b1) -> SBUF bf16  (one op)
  - PE mm2 x4: l(128n x 2) = Hr_sub.T @ [W2|0] -> one LP (128 x 8)
  - ACT: e(128n x 4) = exp(l + b2) -> SBUF (one strided op)
  - DVE/Pool x4: A(128n x 128s) = (iota == bvec) * e  (one tensor_scalar)
  - PE mm_seg x4: S(128s x 258) += A_sub.T @ xe_tile (PSUM acc;
    xe col 256 is the ones column -> per-segment sum of e)
Tail: S -> SBUF, per-segment normalize with host 1/cnt terms, DMA out
(128 x 256) rows = segments [128c, 128(c+1)).
"""
import numpy as np
from contextlib import ExitStack

import ml_dtypes

import concourse.bass as bass
import concourse.bacc as bacc
import concourse.mybir as mybir
import concourse.tile as tile
from concourse.bass_utils import run_bass_kernel_spmd

F32 = mybir.dt.float32
BF16 = mybir.dt.bfloat16
FP8 = mybir.dt.float8e4

NCORES = 8
D = 256
B = 1024
SEG_PER_CORE = B // NCORES  # 128
MAC = 4  # tiles per macro
DMAC = 2  # macros per dma_start


def build_nc(T, num_devices=NCORES):
    """Build the SPMD program for T 128-node tiles per core."""
    nc = bacc.Bacc("TRN2", target_bir_lowering=False, debug=False,
                   enable_asserts=False, num_devices=num_devices)

    xe_ext = nc.dram_tensor("xe", [128, T * 258], BF16, kind="ExternalInput")
    xb_ext = nc.dram_tensor("xb", [128, T * 256], BF16, kind="ExternalInput")
    w1_ext = nc.dram_tensor("w1", [256, 128], BF16, kind="ExternalInput")
    w2p_ext = nc.dram_tensor("w2p", [128, 2], BF16, kind="ExternalInput")
    b1_ext = nc.dram_tensor("b1c", [128, 1], F32, kind="ExternalInput")
    b2_ext = nc.dram_tensor("b2c", [128, 1], F32, kind="ExternalInput")
    mask_ext = nc.dram_tensor("maskw", [128, 128 * T], FP8, kind="ExternalInput")
    inv1_ext = nc.dram_tensor("inv1c", [128, 1], F32, kind="ExternalInput")
    inv2_ext = nc.dram_tensor("inv2c", [128, 1], F32, kind="ExternalInput")
    out_ext = nc.dram_tensor("out", [128, 256], F32, kind="ExternalOutput")

    with tile.TileContext(nc) as tc, ExitStack() as ctx:
        const = ctx.enter_context(tc.tile_pool(name="const", bufs=1))
        xpool = ctx.enter_context(tc.tile_pool(name="xpool", bufs=4))
        xbpool = ctx.enter_context(tc.tile_pool(name="xbpool", bufs=4))
        mpool = ctx.enter_context(tc.tile_pool(name="mpool", bufs=4))
        hrp = ctx.enter_context(tc.tile_pool(name="hrp", bufs=5))
        ep = ctx.enter_context(tc.tile_pool(name="ep", bufs=5))
        ap_pool = ctx.enter_context(tc.tile_pool(name="ap", bufs=5))
        tailp = ctx.enter_context(tc.tile_pool(name="tailp", bufs=1))
        htps = ctx.enter_context(tc.tile_pool(name="htps", bufs=3,
                                              space="PSUM"))
        lps = ctx.enter_context(tc.tile_pool(name="lps", bufs=2, space="PSUM"))
        sps = ctx.enter_context(tc.tile_pool(name="sps", bufs=1, space="PSUM"))

        # --- constants ---
        w1a = const.tile([128, 128], BF16)
        nc.sync.dma_start(w1a[:], w1_ext[0:128, :])
        w1b = const.tile([128, 128], BF16)
        nc.sync.dma_start(w1b[:], w1_ext[128:256, :])
        w2p = const.tile([128, 2], BF16)
        nc.sync.dma_start(w2p[:], w2p_ext[:, :])
        b1c = const.tile([128, 1], F32)
        nc.sync.dma_start(b1c[:], b1_ext[:, :])
        b2c = const.tile([128, 1], F32)
        nc.sync.dma_start(b2c[:], b2_ext[:, :])
        inv1c = const.tile([128, 1], F32)
        nc.sync.dma_start(inv1c[:], inv1_ext[:, :])
        inv2c = const.tile([128, 1], F32)
        nc.sync.dma_start(inv2c[:], inv2_ext[:, :])

        S_ps = sps.tile([128, 258], F32)

        assert T % (MAC * DMAC) == 0
        nmac = T // MAC
        total = nmac
        # Stage lags (in macro ticks) for software pipelining.
        LAG_MM1, LAG_MM2, LAG_SEG = 1, 2, 3
        state = {}
        for vi in range(total + LAG_SEG + 1):
            i = vi if vi < total else -1
            if vi < total:
                st = state.setdefault(vi, {})
                if i % DMAC == 0:
                    XM = xpool.tile([128, 258 * MAC * DMAC], BF16, tag="x")
                    nc.sync.dma_start(
                        XM[:], xe_ext[:, 258 * MAC * i:258 * MAC * (i + DMAC)])
                    XB = xbpool.tile([128, 256 * MAC * DMAC], BF16, tag="xb")
                    nc.sync.dma_start(
                        XB[:], xb_ext[:, 256 * MAC * i:256 * MAC * (i + DMAC)])
                    MW = mpool.tile([128, 128 * MAC * DMAC], FP8, tag="m")
                    nc.sync.dma_start(
                        MW[:], mask_ext[:, 128 * MAC * i:128 * MAC * (i + DMAC)])
                    state["XM"], state["XB"], state["MW"] = XM, XB, MW
                else:
                    XM, XB, MW = state["XM"], state["XB"], state["MW"]
                h = (i % DMAC) * MAC
                st["X"] = [XM[:, 258 * (h + j):258 * (h + j + 1)]
                           for j in range(MAC)]
                st["XBv"] = XB[:, 1024 * (i % DMAC):1024 * (i % DMAC + 1)]
                st["MWv"] = MW[:, 128 * MAC * (i % DMAC):
                               128 * MAC * (i % DMAC + 1)]

            k = vi - LAG_MM1
            if 0 <= k < total:
                st = state[k]
                xbv = st["XBv"]
                HT = htps.tile([128, 512], F32)
                nc.tensor.matmul(HT[:], w1a[:], xbv[:, 0:512],
                                 start=True, stop=False)
                nc.tensor.matmul(HT[:], w1b[:], xbv[:, 512:1024],
                                 start=False, stop=True)
                HR = hrp.tile([128, 512], BF16)
                nc.scalar.activation(HR[:], HT[:],
                                     mybir.ActivationFunctionType.Relu,
                                     bias=b1c[:])
                st["HR"] = HR

            k = vi - LAG_MM2
            if 0 <= k < total:
                st = state[k]
                HR = st["HR"]
                LP = lps.tile([128, 2 * MAC], F32)
                for j in range(MAC):
                    nc.tensor.matmul(LP[:, 2 * j:2 * j + 2],
                                     HR[:, 128 * j:128 * (j + 1)], w2p[:],
                                     start=True, stop=True,
                                     skip_group_check=True)
                E4 = ep.tile([128, MAC], F32)
                nc.scalar.activation(E4[:], LP[:, 0:2 * MAC:2],
                                     mybir.ActivationFunctionType.Exp,
                                     bias=b2c[:])
                A2 = ap_pool.tile([128, 128 * MAC], BF16, tag="A")
                m_in = st["MWv"].rearrange("p (t n) -> p t n", t=MAC)
                e_in = E4[:, :, None].broadcast_to([128, MAC, 128])
                nc.vector.tensor_tensor(
                    out=A2[:].rearrange("p (t n) -> p t n", t=MAC),
                    in0=m_in, in1=e_in, op=mybir.AluOpType.mult)
                st["A2"] = A2

            k = vi - LAG_SEG
            if 0 <= k < total:
                st = state.pop(k)
                A2 = st["A2"]
                for j in range(MAC):
                    nc.tensor.matmul(S_ps[:, :],
                                     A2[:, 128 * j:128 * (j + 1)],
                                     st["X"][j][:, :],
                                     start=(k == 0 and j == 0),
                                     stop=(k == total - 1 and j == MAC - 1),
                                     skip_group_check=True)

        # ---- tail: normalize ----
        S_sb = tailp.tile([128, 258], F32)
        nc.vector.tensor_copy(S_sb[:], S_ps[:])
        dvec = tailp.tile([128, 1], F32)
        nc.vector.tensor_scalar(out=dvec[:], in0=S_sb[:, 256:257],
                                scalar1=inv1c[:], scalar2=1e-8,
                                op0=mybir.AluOpType.mult,
                                op1=mybir.AluOpType.add)
        rvec = tailp.tile([128, 1], F32)
        nc.vector.reciprocal(rvec[:], dvec[:])
        scl = tailp.tile([128, 1], F32)
        nc.vector.tensor_mul(scl[:], rvec[:], inv2c[:])
        out_sb = tailp.tile([128, 256], F32)
        nc.vector.tensor_scalar(out=out_sb[:], in0=S_sb[:, 0:256],
                                scalar1=scl[:], scalar2=None,
                                op0=mybir.AluOpType.mult)
        nc.sync.dma_start(out_ext[:, :], out_sb[:])

    nc.compile()
    return nc


def host_prep(x, batch, W1, b1, W2, b2, ncores=NCORES):
    """Shard on segment boundaries and build per-core input maps."""
    x = np.ascontiguousarray(np.asarray(x, dtype=np.float32))
    batch = np.asarray(batch).astype(np.int64)
    W1 = np.asarray(W1, dtype=np.float32)
    b1 = np.asarray(b1, dtype=np.float32)
    W2 = np.asarray(W2, dtype=np.float32)
    b2 = np.asarray(b2, dtype=np.float32)
    N = x.shape[0]

    sizes = np.bincount(batch, minlength=B)
    starts = np.zeros(B + 1, np.int64)
    starts[1:] = np.cumsum(sizes)
    spc = B // ncores

    T = 0
    for c in range(ncores):
        n = int(starts[spc * (c + 1)] - starts[spc * c])
        T = max(T, (n + 127) // 128)
    T += (-T) % (MAC * DMAC)

    w1_np = W1.astype(ml_dtypes.bfloat16)
    w2p_np = np.zeros((128, 2), ml_dtypes.bfloat16)
    w2p_np[:, 0] = W2[:, 0].astype(ml_dtypes.bfloat16)
    b1_np = b1.reshape(128, 1).astype(np.float32)
    b2_np = np.full((128, 1), float(np.asarray(b2).reshape(-1)[0]), np.float32)
    in_maps = []
    for c in range(ncores):
        lo, hi = int(starts[spc * c]), int(starts[spc * (c + 1)])
        n = hi - lo
        xb16 = x[lo:hi].astype(ml_dtypes.bfloat16)

        xe = np.zeros((T * 128, 258), ml_dtypes.bfloat16)
        xe[:n, 0:256] = xb16
        xe[:, 256] = 1.0
        # swizzle: partition p holds its own rows contiguously
        xe = np.ascontiguousarray(
            xe.reshape(T, 128, 258).transpose(1, 0, 2).reshape(128, T * 258))

        nm = T // MAC
        xbm = np.zeros((256, T * 128), ml_dtypes.bfloat16)
        xbm[:, :n] = xb16.T
        # (c p) (i n) -> p (i c n)
        xbm = np.ascontiguousarray(
            xbm.reshape(2, 128, nm, 512).transpose(1, 2, 0, 3).reshape(
                128, T * 256))

        local = (batch[lo:hi] - spc * c).astype(np.int64)
        assert n == 0 or (local.min() >= 0 and local.max() < spc)

        # one-hot mask: maskw[p, 128*t + s] = 1 iff node (t*128+p) in seg s
        maskw = np.zeros((128, 128 * T), ml_dtypes.float8_e4m3)
        pos = np.arange(n)
        maskw[pos % 128, 128 * (pos // 128) + local] = 1.0

        cnt = sizes[spc * c: spc * (c + 1)].astype(np.float32)
        cntc = np.maximum(cnt, 1.0)
        inv1 = (np.float32(N) / cntc).reshape(128, 1).astype(np.float32)
        inv2 = (1.0 / cntc).reshape(128, 1).astype(np.float32)

        im = {
            "xe": xe,
            "xb": xbm,
            "w1": w1_np,
            "w2p": w2p_np,
            "b1c": b1_np,
            "b2c": b2_np,
            "maskw": maskw,
            "inv1c": inv1,
            "inv2c": inv2,
        }
        in_maps.append(im)
    return T, in_maps


_NC_CACHE = {}
_LAST_RESULTS = None
RUN_KWARGS = {}


def kernel(x, batch, W1, b1, W2, b2):
    global _LAST_RESULTS
    import os
    T, in_maps = host_prep(x, batch, W1, b1, W2, b2)
    key = T
    if key not in _NC_CACHE:
        _NC_CACHE[key] = build_nc(T)
    nc = _NC_CACHE[key]
    kw = dict(RUN_KWARGS)
    if os.environ.get("BASS_KERNEL_TRACE"):
        kw.setdefault("trace", True)
    res = run_bass_kernel_spmd(nc, in_maps, list(range(NCORES)), **kw)
    _LAST_RESULTS = res
    out = np.concatenate([res.results[c]["out"] for c in range(NCORES)], axis=0)
    return out.astype(np.float32)
